# revision 1
# baseline (speedup 1.0000x reference)
"""AutoCorrelation (FFT cross-correlation + full-sort delay aggregation) on 8
NeuronCores, pure data parallel over batch.

Math (per batch b, channels c = (h,e), C = 512, L = 512):
  mv[t]   = (1/C) sum_c irfft( Q_c * conj(K_c) )[t]        (channel-mean correlation)
  rank0   = descending ranks of mv[batch 0]
  g[b, j] = softmax(mv[b])_sorted[ rank0[j] ]              (rank-matched scatter)
  out[b,t,c] = sum_u g[b,u] * v[b,(t+u) % L, c]            (circular correlation)

Layout: 4 local batches per core. Batch-0's shared delay ranks are built
cooperatively: each core computes batch-0 cross-spectra over its 64-channel
slice (1/8 of the FFT work) and AllGathers the 5 reduced spectral columns
(4 KB) early so the rank stage overlaps the local FFT pipeline.

Kernel structure per batch slot:
  - rfft of q,k as bf16 matmuls against a packed [512,512] DFT constant
    (cos block | Nyquist-in-sin-col-0 trick), host casts inputs to bf16
  - spectra products + channel reduction via DVE scalar_tensor_tensor with
    the q-spectrum read straight from PSUM against a bf16 SBUF k-copy
  - irfft as fp32r matvecs (full PE rate at 512-wide output, no LD_WEIGHTS)
  - per-position rank codes via is_gt-count (DVE) / Sign-accumulate (ACT)
    split two columns each; codes and equality masks stay f32 because they
    are exact integers that bf16 cannot represent
  - g assembled by equality-mask matvecs, circulant of g materialized via a
    doubled bf16 row in DRAM read back with a 1023-stride diagonal DMA
  - delay aggregation as bf16 circulant matmuls, bf16 output (host upcasts)

Emission order is the schedule (per-engine queues are in-order): the five
FFT pipelines are software-pipelined so each slot's irfft/rank work hides
behind the next slot's FFT matmuls, and the per-slot mask->g->circulant
chains + stage-C matmul blocks are interleaved by measured readiness.
Loads/stores are fused wide DMAs to amortize the 565 ns SP dispatch cost.
"""

import sys, os
for _p in ('/opt/trn_rl_repo',):
    if _p not in sys.path:
        sys.path.insert(0, _p)

import numpy as np
import ml_dtypes
from contextlib import ExitStack

import concourse.bass as bass
import concourse.bacc as bacc
import concourse.tile as tile
import concourse.mybir as mybir
from concourse.bass_utils import run_bass_kernel_spmd

F32 = mybir.dt.float32
F32R = mybir.dt.float32r
BF16 = mybir.dt.bfloat16
AL = mybir.AluOpType
AF = mybir.ActivationFunctionType
BF = ml_dtypes.bfloat16

B, L, H, E = 32, 512, 8, 64
C = H * E          # 512 channels per batch
NCORES = 8
NB = B // NCORES   # 4 local batches per core
CP = C // NCORES   # 64 batch-0 channels per core


def _consts():
    l = np.arange(L)[:, None].astype(np.float64)
    f = np.arange(257)[None, :].astype(np.float64)
    Wc = np.cos(2 * np.pi * l * f / L)            # [512, 257]
    Ws = np.sin(2 * np.pi * l * f[:, :256] / L)   # [512, 256]
    # W2 = [cos f=0..255 | {Nyquist cos, sin f=1..255}] : [512, 512]
    Ws2 = Ws.copy()
    Ws2[:, 0] = np.cos(np.pi * np.arange(L))
    W2 = np.concatenate([Wc[:, 0:256], Ws2], axis=1).astype(BF)

    m = np.arange(L)[None, :].astype(np.float64)
    fc = np.arange(257)[:, None].astype(np.float64)
    wgt = np.where((fc == 0) | (fc == 256), 1.0, 2.0)
    Ar = (wgt * np.cos(2 * np.pi * fc * m / L) / (L * C))             # [257, 512]
    fs = np.arange(256)[:, None].astype(np.float64)
    wgt_i = np.where(fs == 0, 0.0, 2.0)
    Ai = (-wgt_i * np.sin(2 * np.pi * fs * m / L) / (L * C))          # [256, 512]
    ArB0 = Ar[0:128].copy()
    ArB0[0] = Ar[256]   # Nyquist row pairs with the sin-half partition-0 product
    # irfft rhs pack: [Ar0, ArB0, Ar1, Ai0, Ai1] each [128, 512]
    ArAi = np.stack([Ar[0:128], ArB0, Ar[128:256], Ai[0:128], Ai[128:256]],
                    axis=0).astype(BF)                                 # [5,128,512]

    # packed small consts: [antiI | two | one] -> [128, 130] f32
    small = np.zeros((128, 130), np.float32)
    small[np.arange(128), 127 - np.arange(128)] = 1.0   # anti-identity
    small[:, 128] = 2.0
    small[0, 129] = 1.0
    return W2, ArAi, small


_NC_CACHE = None


def _build():
    global _NC_CACHE
    if _NC_CACHE is not None:
        return _NC_CACHE
    W2_np, ArAi_np, small_np = _consts()

    nc = bacc.Bacc("TRN2", target_bir_lowering=False, debug=False, num_devices=NCORES)
    tc = tile.TileContext(nc)

    q_all = nc.dram_tensor("q_all", [NB, L, C], BF16, kind="ExternalInput")
    k_all = nc.dram_tensor("k_all", [NB, L, C], BF16, kind="ExternalInput")
    qk0p = nc.dram_tensor("qk0p", [L, 2 * CP], BF16, kind="ExternalInput")
    v_all = nc.dram_tensor("v_all", [NB, L, C], BF16, kind="ExternalInput")
    out_all = nc.dram_tensor("out_all", [NB, L, C], BF16, kind="ExternalOutput")

    W2_d = nc.inline_tensor(W2_np, "W2_d")
    ArAi_d = nc.inline_tensor(ArAi_np, "ArAi_d")
    small_d = nc.inline_tensor(small_np, "small_d")

    # [128, lc*512 + c] <- dram [slot, 128*lc + p, c]
    def ap_lc(tensor, slot):
        return bass.AP(tensor=tensor, offset=slot * L * C,
                       ap=[[C, 128], [128 * C, 4], [1, C]])

    with tc, ExitStack() as ctx:
        cpool = ctx.enter_context(tc.tile_pool(name="consts", bufs=1))
        iopool = ctx.enter_context(tc.tile_pool(name="io", bufs=1))
        wpool = ctx.enter_context(tc.tile_pool(name="work", bufs=1))
        spool = ctx.enter_context(tc.tile_pool(name="scol", bufs=1))
        pspec = ctx.enter_context(tc.tile_pool(name="pspec", bufs=1, space="PSUM"))
        psmall = ctx.enter_context(tc.tile_pool(name="psmall", bufs=1, space="PSUM"))
        dpool = ctx.enter_context(tc.tile_pool(name="dscratch", bufs=1, space="DRAM"))

        # ---- constants ----
        W2sb = cpool.tile([128, 2048], BF16, name="W2sb")
        ArAisb = cpool.tile([128, 2560], F32R, name="ArAisb")
        ArAibf = cpool.tile([128, 2560], BF16, name="ArAibf")
        small_t = cpool.tile([128, 130], F32, name="small_t")
        antiI_t = small_t[:, 0:128]
        two_t = small_t[:, 128:129].bitcast(F32R)
        one_t = small_t[0:1, 129:130]
        n2bBR = cpool.tile([128, 512], F32, name="n2bBR")      # 2*cnt_gt code
        n2bBR_s = cpool.tile([128, 512], F32, name="n2bBR_s")  # ranksign code
        late = [False]

        def load_late_consts():
            if late[0]:
                return
            late[0] = True
            nc.sync.dma_start(small_t[:], small_d.ap())
            nc.sync.dma_start(ArAibf[:], bass.AP(tensor=ArAi_d, offset=0,
                                                 ap=[[512, 128], [512 * 128, 5], [1, 512]]))
            nc.vector.tensor_copy(ArAisb[:], ArAibf[:])

        # ---------- stage A: DMA + FFT + spectra products ----------
        def stage_fft(s, chunked=False):
            qsb = iopool.tile([128, 2048], BF16, name=f"q_s{s}", tag="qt", bufs=4)
            ksb = iopool.tile([128, 2048], BF16, name=f"k_s{s}", tag="kt", bufs=4)
            if chunked:
                for lc in range(4):
                    nc.sync.dma_start(ksb[:, 512 * lc:512 * (lc + 1)],
                                      bass.AP(tensor=k_all, offset=s * L * C + 128 * lc * C,
                                              ap=[[C, 128], [1, C]]))
                for h in range(2):
                    nc.sync.dma_start(qsb[:, 1024 * h:1024 * (h + 1)],
                                      bass.AP(tensor=q_all, offset=s * L * C + 256 * h * C,
                                              ap=[[C, 128], [128 * C, 2], [1, C]]))
                load_late_consts()
            else:
                nc.sync.dma_start(qsb[:], ap_lc(q_all, s))
                nc.sync.dma_start(ksb[:], ap_lc(k_all, s))

            scolp = spool.tile([128, 8], F32R, name=f"scolp_s{s}", tag="scolp", bufs=2)
            for g in range(2):
                sq = pspec.tile([128, 1024], F32, name=f"specq_s{s}g{g}", tag="spec", bufs=3)
                sk = pspec.tile([128, 1024], F32, name=f"speck_s{s}g{g}", tag="spec", bufs=3)
                sk_sb = wpool.tile([128, 1024], BF16, name=f"sksb_s{s}g{g}", tag="sksb", bufs=2)
                for (spec, src) in ((sk, ksb), (sq, qsb)):
                    for lc in range(4):
                        nc.tensor.matmul(spec[:, 0:512],
                                         W2sb[:, 512 * lc + 128 * g: 512 * lc + 128 * g + 128],
                                         src[:, 512 * lc: 512 * (lc + 1)],
                                         start=(lc == 0), stop=(lc == 3))
                    for lc in range(4):
                        nc.tensor.matmul(spec[:, 512:1024],
                                         W2sb[:, 512 * lc + 256 + 128 * g: 512 * lc + 256 + 128 * g + 128],
                                         src[:, 512 * lc: 512 * (lc + 1)],
                                         start=(lc == 0), stop=(lc == 3))
                    if spec is sk:
                        nc.scalar.copy(sk_sb[:], sk[:])
                scr = wpool.tile([128, 1024], BF16, name=f"sttscr_s{s}g{g}", tag="sttscr", bufs=2)
                if g == 0:
                    # cos-half and sin-half reduced separately: sin-half partition 0
                    # carries the Nyquist product and pairs with ArB0's Nyquist row.
                    nc.vector.scalar_tensor_tensor(scr[:, 0:512], sq[:, 0:512], 1.0,
                                                   sk_sb[:, 0:512], AL.mult, AL.mult,
                                                   accum_out=scolp[:, 0:1])
                    nc.vector.scalar_tensor_tensor(scr[:, 512:1024], sq[:, 512:1024], 1.0,
                                                   sk_sb[:, 512:1024], AL.mult, AL.mult,
                                                   accum_out=scolp[:, 1:2])
                else:
                    nc.vector.scalar_tensor_tensor(scr[:], sq[:], 1.0,
                                                   sk_sb[:], AL.mult, AL.mult,
                                                   accum_out=scolp[:, 2:3])
                # S_i = sum_c Qr*Ks - Qs*Kr  (f=0 garbage killed by Ai's zero row)
                nc.vector.scalar_tensor_tensor(scr[:, 0:512], sq[:, 0:512], 1.0,
                                               sk_sb[:, 512:1024], AL.mult, AL.mult,
                                               accum_out=scolp[:, 5:6])
                nc.vector.scalar_tensor_tensor(scr[:, 0:512], sq[:, 512:1024], 1.0,
                                               sk_sb[:, 0:512], AL.mult, AL.mult,
                                               accum_out=scolp[:, 6:7])
                nc.vector.tensor_sub(scolp[:, 3 + g:4 + g], scolp[:, 5:6].bitcast(F32), scolp[:, 6:7].bitcast(F32))
            return scolp

        # ---------- stage B: irfft -> mv, transposed mv ----------
        def finish_core(s, scolp, with_v=True):
            mv_ps = psmall.tile([1, 512], F32, name=f"mvps_s{s}", tag="psm", bufs=2)
            for i in range(5):
                nc.tensor.matmul(mv_ps[:], scolp[:, i:i + 1].bitcast(F32R),
                                 ArAisb[:, 512 * i: 512 * (i + 1)],
                                 start=(i == 0), stop=(i == 4))
            mv_sb = wpool.tile([1, 512], F32, name=f"mvsb_s{s}", tag="mvsb", bufs=5)
            nc.scalar.copy(mv_sb[:], mv_ps[:])
            mvB = wpool.tile([128, 512], F32, name=f"mvB_s{s}", tag="mvB", bufs=2)
            nc.gpsimd.partition_broadcast(mvB[:], mv_sb[:])
            mvT_ps = psmall.tile([128, 4], F32, name=f"mvT_s{s}", tag="psm", bufs=2)
            for j in range(4):
                nc.tensor.transpose(mvT_ps[:, j:j + 1], mv_sb[0:1, 128 * j:128 * (j + 1)], one_t)
            mvT_sb = wpool.tile([128, 4], F32, name=f"mvTsb_s{s}", tag="mvTsb", bufs=5)
            nc.scalar.copy(mvT_sb[:], mvT_ps[:])
            if with_v:
                vsb = iopool.tile([128, 2048], BF16, name=f"v_s{s}", tag="vt", bufs=4)
                nc.sync.dma_start(vsb[:], ap_lc(v_all, s))
                v_res[s] = vsb
            return mv_sb, mvB, mvT_sb

        # ---------- rank code + softmax pieces ----------
        def finish_code(s):
            mv_sb, mvB, mvT_sb = slot_res[s]
            # per-position descending-rank code. Two equivalent encodings:
            # DVE computes 2*cnt_gt[i] (matches n2bBR), ACT computes
            # ranksign[i] = 2*cnt_gt[i] - 511 (matches n2bBR_s). mv values
            # are distinct f32 so both are exact integers.
            rs = wpool.tile([128, 4], F32, name=f"rs_{s}", tag="rs", bufs=4)
            sgnscr = wpool.tile([128, 512], F32, name=f"sgnscr_{s}", tag="sgnscr", bufs=2)
            sgnscr2 = wpool.tile([128, 512], F32, name=f"sgnscr2_{s}", tag="sgnscr2", bufs=2)
            negmvT = wpool.tile([128, 4], F32, name=f"negmvT_{s}", tag="negmvT", bufs=2)
            nc.scalar.activation(negmvT[:], mvT_sb[:], AF.Copy, bias=0.0, scale=-1.0)
            for j in range(2):
                nc.vector.tensor_scalar(sgnscr[:], mvB[:], mvT_sb[:, j:j + 1], None,
                                        AL.is_gt, AL.add, accum_out=rs[:, j:j + 1])
            for j in range(2, 4):
                nc.scalar.activation(sgnscr2[:], mvB[:], AF.Sign, bias=negmvT[:, j:j + 1],
                                     accum_out=rs[:, j:j + 1])
            expz = wpool.tile([1, 512], F32, name=f"expz_{s}", tag="expz", bufs=2)
            z_sb = wpool.tile([1, 1], F32, name=f"z_{s}", tag="z", bufs=4)
            nc.scalar.activation(expz[:], mv_sb[:], AF.Exp, accum_out=z_sb[:])
            rz = wpool.tile([1, 1], F32, name=f"rz_{s}", tag="rz", bufs=4)
            nc.vector.reciprocal(rz[:], z_sb[:])
            smc = wpool.tile([128, 4], F32, name=f"smc_{s}", tag="smc", bufs=4)
            nc.scalar.activation(smc[:], mvT_sb[:], AF.Exp)
            smcr = wpool.tile([128, 4], F32R, name=f"smcr_{s}", tag="smcr", bufs=4)
            nc.vector.tensor_copy(smcr[:], smc[:])
            code_res[s] = (rs, rz, smcr)

        # ---------- stage C chain: equality masks -> g -> circulant DMA ----------
        chain_res = {}


        # ---------- batch-0 partial spectra + AllGather ----------
        def emit_partial():
            qk = iopool.tile([128, 512], BF16, name="qk0sb", bufs=1)
            nc.sync.dma_start(qk[:], bass.AP(tensor=qk0p, offset=0,
                                             ap=[[2 * CP, 128], [128 * 2 * CP, 4], [1, 2 * CP]]))
            nc.sync.dma_start(W2sb[:, 0:512],
                              bass.AP(tensor=W2_d, offset=0, ap=[[512, 128], [1, 512]]))
            for lc in range(1, 4):
                nc.sync.dma_start(W2sb[:, 512 * lc:512 * (lc + 1)],
                                  bass.AP(tensor=W2_d, offset=128 * lc * 512,
                                          ap=[[512, 128], [1, 512]]))
            sp_ps = psmall.tile([128, 512], F32, name="sp0_ps", tag="psm", bufs=2)
            for g in range(2):
                for half in range(2):
                    for lc in range(4):
                        nc.tensor.matmul(
                            sp_ps[:, 256 * g + 128 * half: 256 * g + 128 * half + 128],
                            W2sb[:, 512 * lc + 256 * half + 128 * g: 512 * lc + 256 * half + 128 * g + 128],
                            qk[:, 128 * lc: 128 * (lc + 1)],
                            start=(lc == 0), stop=(lc == 3))
            sp = wpool.tile([128, 512], F32, name="sp0_sb", bufs=1)
            nc.scalar.copy(sp[:], sp_ps[:])
            qc = lambda g: sp[:, 256 * g + 0: 256 * g + CP]
            kc = lambda g: sp[:, 256 * g + CP: 256 * g + 2 * CP]
            qs = lambda g: sp[:, 256 * g + 128: 256 * g + 128 + CP]
            ks = lambda g: sp[:, 256 * g + 128 + CP: 256 * g + 128 + 2 * CP]
            scolp = spool.tile([128, 8], F32R, name="scolp0", bufs=1)
            scr0 = wpool.tile([128, 128], F32, name="scr0", bufs=1)
            nc.vector.scalar_tensor_tensor(scr0[:, 0:CP], qc(0), 1.0, kc(0),
                                           AL.mult, AL.mult, accum_out=scolp[:, 0:1])
            nc.vector.scalar_tensor_tensor(scr0[:, 0:CP], qs(0), 1.0, ks(0),
                                           AL.mult, AL.mult, accum_out=scolp[:, 1:2])
            nc.vector.scalar_tensor_tensor(scr0[:, 0:CP], qc(1), 1.0, kc(1),
                                           AL.mult, AL.mult, accum_out=scolp[:, 2:3])
            nc.vector.scalar_tensor_tensor(scr0[:, 0:CP], qs(1), 1.0, ks(1),
                                           AL.mult, AL.mult, accum_out=scolp[:, 7:8])
            nc.vector.tensor_tensor(scolp[:, 2:3], scolp[:, 2:3].bitcast(F32), scolp[:, 7:8].bitcast(F32), AL.add)
            for g in range(2):
                nc.vector.scalar_tensor_tensor(scr0[:, 0:CP], qc(g), 1.0, ks(g),
                                               AL.mult, AL.mult, accum_out=scolp[:, 5:6])
                nc.vector.scalar_tensor_tensor(scr0[:, 0:CP], qs(g), 1.0, kc(g),
                                               AL.mult, AL.mult, accum_out=scolp[:, 6:7])
                nc.vector.tensor_sub(scolp[:, 3 + g:4 + g], scolp[:, 5:6].bitcast(F32), scolp[:, 6:7].bitcast(F32))
            return scolp

        def emit_partial_cc(scolp):
            scol_d = dpool.tile([128, 8], F32, name="scol_d", bufs=1)
            nc.sync.dma_start(scol_d[:], scolp[:].bitcast(F32))
            scolg_d = dpool.tile([NCORES * 128, 8], F32, name="scolg_d", bufs=1)
            nc.gpsimd.collective_compute(
                "AllGather", AL.bypass, [list(range(NCORES))],
                ins=[scol_d[:]], outs=[scolg_d[:]])
            return scolg_d

        def gather_sum(scolg_d):
            scolg = spool.tile([128, 64], F32, name="scolg_sb", bufs=1)
            nc.sync.dma_start(scolg[:], bass.AP(tensor=scolg_d[:].tensor, offset=0,
                                                ap=[[8, 128], [128 * 8, 8], [1, 8]]))
            acc = spool.tile([128, 8], F32R, name="scolacc", bufs=1)
            nc.vector.tensor_tensor(acc[:], scolg[:, 0:8], scolg[:, 8:16], AL.add)
            for r in range(2, NCORES):
                nc.vector.tensor_tensor(acc[:], acc[:].bitcast(F32), scolg[:, 8 * r:8 * (r + 1)], AL.add)
            return acc

        wt_res = {}
        v_res = {}
        code_res = {}

        def emit_wt(s):
            rs, rz, smcr = code_res[s]
            wts = []
            for j in range(4):
                code = n2bBR if j < 2 else n2bBR_s
                wt = wpool.tile([128, 512], F32R, name=f"wt_{s}_{j}", tag=f"wt{j}", bufs=4)
                nc.vector.tensor_scalar(wt[:], code[:], rs[:, j:j + 1], None, AL.is_equal)
                wts.append(wt)
            wt_res[s] = wts

        def emit_gchain(s):
            rs, rz, smcr = code_res[s]
            wts = wt_res[s]
            g_ps = psmall.tile([1, 512], F32, name=f"gps_{s}", tag="psm", bufs=2)
            for j in range(4):
                nc.tensor.matmul(g_ps[:], smcr[:, j:j + 1], wts[j][:], start=(j == 0), stop=(j == 3))
            gn = wpool.tile([1, 512], BF16, name=f"gn_{s}", tag="gn", bufs=2)
            nc.vector.tensor_scalar(gn[:], g_ps[:], rz[:], None, AL.mult)
            gRB = wpool.tile([128, 512], BF16, name=f"gRB_{s}", tag="gRB", bufs=2)
            nc.gpsimd.partition_broadcast(gRB[:], gn[:])
            gmat = dpool.tile([128, 1024], BF16, name=f"gmat_{s}", tag="gmat", bufs=2)
            nc.sync.dma_start(gmat[:, 0:512], gRB[:])
            nc.sync.dma_start(gmat[:, 512:1024], gRB[:])
            vsb = v_res[s]
            gd = gmat[:].tensor
            cg = iopool.tile([128, 2048], BF16, name=f"cg_s{s}", tag="cg", bufs=4)
            for ss in range(4):
                nc.sync.dma_start(cg[:, 512 * ss:512 * (ss + 1)],
                                  bass.AP(tensor=gd, offset=511 - 128 * ss,
                                          ap=[[1023, 128], [1, 512]]))
            chain_res[s] = (vsb, cg)

        # ---------- stage C block for one slot ----------
        def emit_stagec(s):
            vsb, cg = chain_res[s]
            o_sb = wpool.tile([128, 2048], BF16, name=f"osb_{s}", tag="osb", bufs=4)
            for tt in range(4):
                o_ps = psmall.tile([128, 512], F32, name=f"ops_{s}_{tt}", tag="psm", bufs=2)
                for ss in range(4):
                    nc.tensor.matmul(o_ps[:], cg[:, 512 * ss + 128 * tt: 512 * ss + 128 * tt + 128],
                                     vsb[:, 512 * ss: 512 * (ss + 1)],
                                     start=(ss == 0), stop=(ss == 3))
                nc.scalar.copy(o_sb[:, 512 * tt: 512 * (tt + 1)], o_ps[:])
                nc.sync.dma_start(
                    bass.AP(tensor=out_all, offset=s * L * C + 128 * tt * C,
                            ap=[[C, 128], [1, C]]),
                    o_sb[:, 512 * tt: 512 * (tt + 1)])

        # ================= phase 1 (software-pipelined) =================
        slot_res = {}
        order = list(range(NB))
        pend = None   # (slot, scolp)
        scolg_d = None
        for idx, s in enumerate(order):
            if s == 0:
                scolp0 = emit_partial()
            scolp = stage_fft(s, chunked=(s == 0))
            if s == 0:
                scolg_d = emit_partial_cc(scolp0)
            if pend is not None:
                ps, psc = pend
                slot_res[ps] = finish_core(ps, psc)
                finish_code(ps)
            pend = (s, scolp)
            if idx == 3:
                emit_wt(1)
                emit_gchain(1)
            if idx == 2:
                # batch-0 rank stage: gathered spectra land mid-phase
                acc = gather_sum(scolg_d)
                slot_res['B'] = finish_core('B', acc, with_v=False)
                posmvT = slot_res['B'][2]
                mv0r_ps = psmall.tile([1, 512], F32, name="mv0r_ps", tag="psm", bufs=2)
                for j in range(4):
                    nc.tensor.matmul(mv0r_ps[0:1, 128 * (3 - j):128 * (4 - j)],
                                     posmvT[:, j:j + 1], antiI_t, start=True, stop=True)
                mv0r_sb = wpool.tile([1, 512], F32, name="mv0r_sb", bufs=1)
                nc.scalar.copy(mv0r_sb[:], mv0r_ps[:])
                mvB0R = wpool.tile([128, 512], F32, name="mvB0R", bufs=1)
                nc.gpsimd.partition_broadcast(mvB0R[:], mv0r_sb[:])
                r2_ps = psmall.tile([1, 512], F32, name="r2_ps", tag="psm", bufs=2)
                for j in range(4):
                    c2 = wpool.tile([128, 512], F32R, name=f"c2_{j}", tag="c2", bufs=2)
                    nc.vector.tensor_scalar(c2[:], mvB0R[:], posmvT[:, j:j + 1], None, AL.is_lt)
                    nc.tensor.matmul(r2_ps[:], two_t, c2[:], start=(j == 0), stop=(j == 3))
                n2b_row = wpool.tile([1, 512], F32, name="n2b_row", bufs=1)
                nc.scalar.activation(n2b_row[:], r2_ps[:], AF.Copy, bias=0.0, scale=0.5)
                nc.gpsimd.partition_broadcast(n2bBR[:], n2b_row[:])
                n2bs_row = wpool.tile([1, 512], F32, name="n2bs_row", bufs=1)
                nc.scalar.activation(n2bs_row[:], r2_ps[:], AF.Copy, bias=-511.0, scale=1.0)
                nc.gpsimd.partition_broadcast(n2bBR_s[:], n2bs_row[:])
                emit_wt(0)
                emit_gchain(0)
        # tail: last slot's irfft/mv first, then chains/stage-C by readiness
        ps, psc = pend
        slot_res[ps] = finish_core(ps, psc)
        emit_wt(2)
        emit_gchain(2)
        emit_stagec(0)
        finish_code(3)
        emit_wt(3)
        emit_gchain(3)
        emit_stagec(1)
        emit_stagec(2)
        emit_stagec(3)

    nc.compile()
    _NC_CACHE = nc
    return nc


def kernel(queries, keys, values):
    q = np.ascontiguousarray(queries, dtype=np.float32).reshape(B, L, C).astype(BF)
    k = np.ascontiguousarray(keys, dtype=np.float32).reshape(B, L, C).astype(BF)
    v = np.ascontiguousarray(values, dtype=np.float32).reshape(B, L, C).astype(BF)
    nc = _build()
    in_maps = []
    for c in range(NCORES):
        sl = slice(NB * c, NB * (c + 1))
        in_maps.append({
            "q_all": q[sl],
            "k_all": k[sl],
            "v_all": v[sl],
            "qk0p": np.concatenate([q[0][:, CP * c:CP * (c + 1)],
                                    k[0][:, CP * c:CP * (c + 1)]], axis=1),
        })
    res = run_bass_kernel_spmd(nc, in_maps, core_ids=list(range(NCORES)))
    out = np.concatenate([np.asarray(res.results[c]["out_all"]).astype(np.float32)
                          for c in range(NCORES)], axis=0)
    return out.reshape(B, L, H, E)


if __name__ == "__main__":
    rng = np.random.default_rng(0)
    qq = rng.standard_normal((B, L, H, E)).astype(np.float32)
    kk = rng.standard_normal((B, L, H, E)).astype(np.float32)
    vv = rng.standard_normal((B, L, H, E)).astype(np.float32)
    o = kernel(queries=qq, keys=kk, values=vv)
    print(o.shape, o.dtype, np.abs(o).max())



# revision 4
# speedup vs baseline: 1.1343x; 1.1343x over previous
"""AutoCorrelation (channel-mean circular cross-correlation + rank-matched
delay aggregation) on 8 NeuronCores, pure data parallel over batch.

Math (per batch b, channels c = (h,e), C = 512, L = 512):
  mv[tau]  = (1/C) sum_c sum_t q_c[t] k_c[(t-tau) % L]    (= mean irfft(Q conj K))
  rank0    = descending ranks of mv[batch 0]
  g[b, j]  = softmax(mv[b])_sorted[ rank0[j] ]            (rank-matched scatter)
  out[b,t,c] = sum_u g[b,u] v[b,(t+u) % L, c]             (circular correlation)

Key change vs the FFT formulation: mv is computed WITHOUT any FFT.
  M'[t,s] = sum_c k[t,c] q[s,c]  (one 512^3 bf16 matmul per batch — half the
            PE work of the two DFT matmuls, and no DVE spectra products)
  mv[tau] = (1/C) sum_t M'[t, (t+tau) % L]
The diagonal sum is done by accumulating the four 128-row blocks of M' into
one PSUM tile with per-block column rotations (free: column-sliced matmul
outputs), round-tripping the folded P[128,512] through DRAM with a
(row-stride+1) diagonal read, and one ones-column matvec.

delays[0] comes from batch 0 only: instead of a collective (15us fixed cost
in the cost model), every core redundantly computes batch-0's M' block from
a replicated q0/k0 copy (3.4us PE, fully overlapped).

The delay-aggregation circulant is block-circulant with only FOUR distinct
[128,128] stationary blocks, so the stage-C stationary read is [128,512]
(128KB) instead of [128,2048], from a broadcast doubled-g row in DRAM via a
stride-1023 diagonal AP.

Rank codes use a single cnt_gt-256 encoding, exact in bf16 (|code| <= 256),
so the 16 equality masks run in DVE 2x/4x mode and the masks/g matmuls move
bf16. Emission order is the schedule (per-engine queues are in-order): PE
runs warmup -> M-blocks -> rank matvecs -> g matvecs -> stage C with
measured-readiness interleave so it never idles after ramp-up.
"""

import sys
for _p in ('/opt/trn_rl_repo',):
    if _p not in sys.path:
        sys.path.insert(0, _p)

import numpy as np
import ml_dtypes
from contextlib import ExitStack

import concourse.bass as bass
import concourse.bacc as bacc
import concourse.tile as tile
import concourse.mybir as mybir
from concourse.bass_utils import run_bass_kernel_spmd

F32 = mybir.dt.float32
F32R = mybir.dt.float32r
BF16 = mybir.dt.bfloat16
AL = mybir.AluOpType
AF = mybir.ActivationFunctionType
BF = ml_dtypes.bfloat16

B, L, H, E = 32, 512, 8, 64
C = H * E          # 512 channels per batch
NCORES = 8
NB = B // NCORES   # 4 local batches per core

N_WARM = 6         # PE warmup matmuls (p-state ramp while first load lands)


def _consts():
    # packed small consts: [antiI | two | one] -> [128, 130] f32
    small = np.zeros((128, 130), np.float32)
    small[np.arange(128), 127 - np.arange(128)] = 1.0   # anti-identity
    small[:, 128] = 2.0
    small[0, 129] = 1.0
    invc = np.full((128, 1), 1.0 / C, dtype=BF)         # 2^-9, exact in bf16
    return small, invc


_NC_CACHE = None


def _build():
    global _NC_CACHE
    if _NC_CACHE is not None:
        return _NC_CACHE
    small_np, invc_np = _consts()

    nc = bacc.Bacc("TRN2", target_bir_lowering=False, debug=False, num_devices=NCORES)
    tc = tile.TileContext(nc)

    # qk packed [slot, cchunk(4), {q,k}(2), p(128), t(L)] with q/k transposed
    # to [channel, time] on host so channels are the matmul contraction dim.
    qk_all = nc.dram_tensor("qk_all", [NB, 4, 2, 128, L], BF16, kind="ExternalInput")
    qk0_all = nc.dram_tensor("qk0_all", [4, 2, 128, L], BF16, kind="ExternalInput")
    v_all = nc.dram_tensor("v_all", [NB, L, C], BF16, kind="ExternalInput")
    out_all = nc.dram_tensor("out_all", [NB, L, C], BF16, kind="ExternalOutput")

    small_d = nc.inline_tensor(small_np, "small_d")
    invc_d = nc.inline_tensor(invc_np, "invc_d")

    with tc, ExitStack() as ctx:
        cpool = ctx.enter_context(tc.tile_pool(name="consts", bufs=1))
        qpool = ctx.enter_context(tc.tile_pool(name="qk", bufs=1))
        vpool = ctx.enter_context(tc.tile_pool(name="vv", bufs=1))
        wpool = ctx.enter_context(tc.tile_pool(name="work", bufs=1))
        pM = ctx.enter_context(tc.tile_pool(name="pM", bufs=1, space="PSUM"))
        pC = ctx.enter_context(tc.tile_pool(name="pC", bufs=1, space="PSUM"))
        psm = ctx.enter_context(tc.tile_pool(name="psm", bufs=1, space="PSUM"))
        dpool = ctx.enter_context(tc.tile_pool(name="dscratch", bufs=1, space="DRAM"))

        # ---- constants ----
        small_t = cpool.tile([128, 130], F32, name="small_t")
        antiI_t = small_t[:, 0:128]
        two_t = small_t[:, 128:129].bitcast(F32R)
        one_t = small_t[0:1, 129:130]
        invc_t = cpool.tile([128, 1], BF16, name="invc_t")
        nc.sync.dma_start(small_t[:], small_d.ap())
        nc.sync.dma_start(invc_t[:], invc_d.ap())

        # ---- PE warmup: ramp the p-state while the first loads land ----
        # (output goes into the Mps ring; never read, reset by M0's start=True)
        warm_ps = pM.tile([128, 512], F32, name="warm_ps", tag="Mps", bufs=2)
        for w in range(N_WARM):
            nc.tensor.matmul(warm_ps[:, 0:128], antiI_t, small_t[:, 0:128],
                             start=True, stop=True)

        # ---- loads ----
        def load_qk_half(dram, base_off, qksb, h):
            nc.sync.dma_start(
                qksb[:, 2048 * h:2048 * (h + 1)],
                bass.AP(tensor=dram, offset=base_off + h * 2 * 2 * 128 * L,
                        ap=[[L, 128], [128 * L, 4], [1, L]]))

        def load_v(s):
            vsb = vpool.tile([128, 2048], BF16, name=f"v_s{s}", tag="vt", bufs=4)
            nc.sync.dma_start(vsb[:], bass.AP(tensor=v_all, offset=s * L * C,
                                              ap=[[C, 128], [128 * C, 4], [1, C]]))
            return vsb

        # ---- M' = k q^T with rotated fold into P (one PSUM tile) ----
        # M'[t,s] = sum_c k[t,c] q[s,c];  P[p,u] = sum_i M'[128i+p, (u+128i)%512]
        def emit_M(qksb, nm, ccs=(0, 1, 2, 3), M_ps=None):
            if M_ps is None:
                M_ps = pM.tile([128, 512], F32, name=f"M_{nm}", tag="Mps", bufs=2)
            for cc in ccs:
                qb = 1024 * cc
                kb = 1024 * cc + 512
                for i in range(4):
                    lhs = qksb[:, kb + 128 * i: kb + 128 * i + 128]
                    first = (cc == 0 and i == 0)
                    last = (cc == 3 and i == 3)
                    if i == 0:
                        nc.tensor.matmul(M_ps[:, 0:512], lhs, qksb[:, qb:qb + 512],
                                         start=first, stop=False, skip_group_check=True)
                    else:
                        w = 512 - 128 * i
                        nc.tensor.matmul(M_ps[:, 0:w], lhs, qksb[:, qb + 128 * i:qb + 512],
                                         start=False, stop=last, skip_group_check=True)
                        nc.tensor.matmul(M_ps[:, w:512], lhs, qksb[:, qb:qb + 128 * i],
                                         start=False, stop=last, skip_group_check=True)
            return M_ps

        # ---- P diag round trip: ACT copy, 2 writes, shifted read ----
        def emit_Pdiag(M_ps, nm):
            P_sb = wpool.tile([128, 512], BF16, name=f"P_{nm}", tag="Psb", bufs=2)
            nc.scalar.copy(P_sb[:], M_ps[:])
            P_d = dpool.tile([128, 640], BF16, name=f"Pd_{nm}", tag="Pd", bufs=2)
            pd = P_d[:].tensor
            nc.scalar.dma_start(bass.AP(tensor=pd, offset=0, ap=[[640, 128], [1, 512]]),
                                P_sb[:])
            nc.scalar.dma_start(bass.AP(tensor=pd, offset=512, ap=[[640, 128], [1, 128]]),
                                P_sb[:, 0:128])
            R_sb = wpool.tile([128, 512], BF16, name=f"R_{nm}", tag="Rsb", bufs=2)
            nc.scalar.dma_start(R_sb[:], bass.AP(tensor=pd, offset=0, ap=[[641, 128], [1, 512]]))
            return R_sb

        # ---- mv[tau] = (1/C) sum_p R[p, tau] ----
        def emit_mv(R_sb, nm):
            mv_ps = psm.tile([1, 512], F32, name=f"mvps_{nm}", tag="psm", bufs=2)
            nc.tensor.matmul(mv_ps[:], invc_t[:], R_sb[:], start=True, stop=True)
            mv_sb = wpool.tile([1, 512], F32, name=f"mvsb_{nm}", tag="mvsb", bufs=5)
            nc.vector.tensor_copy(mv_sb[:], mv_ps[:])
            return mv_sb

        def emit_mvT(mv_sb, nm):
            mvT_ps = psm.tile([128, 4], F32, name=f"mvTps_{nm}", tag="psT", bufs=2)
            for j in range(4):
                nc.tensor.transpose(mvT_ps[:, j:j + 1], mv_sb[0:1, 128 * j:128 * (j + 1)], one_t)
            mvT_sb = wpool.tile([128, 4], F32, name=f"mvT_{nm}", tag="mvT", bufs=5)
            nc.scalar.copy(mvT_sb[:], mvT_ps[:])
            return mvT_sb

        # ---- per-slot rank codes (cnt_gt - 256 encoding) + softmax pieces ----
        def finish_code(s, mv_sb, mvT_sb):
            mvB = wpool.tile([128, 512], F32, name=f"mvB_{s}", tag="mvB", bufs=2)
            nc.gpsimd.partition_broadcast(mvB[:], mv_sb[:])
            rs = wpool.tile([128, 4], F32, name=f"rs_{s}", tag="rs", bufs=4)
            sgnscr = wpool.tile([128, 512], F32, name=f"sgn_{s}", tag="sgn", bufs=2)
            sgnscr2 = wpool.tile([128, 512], F32, name=f"sgn2_{s}", tag="sgn2", bufs=2)
            negmvT = wpool.tile([128, 4], F32, name=f"negmvT_{s}", tag="negmvT", bufs=2)
            nc.scalar.activation(negmvT[:], mvT_sb[:], AF.Copy, bias=0.0, scale=-1.0)
            for j in range(2):
                nc.vector.tensor_scalar(sgnscr[:], mvB[:], mvT_sb[:, j:j + 1], None,
                                        AL.is_gt, AL.add, accum_out=rs[:, j:j + 1])
            for j in range(2, 4):
                nc.scalar.activation(sgnscr2[:], mvB[:], AF.Sign, bias=negmvT[:, j:j + 1],
                                     accum_out=rs[:, j:j + 1])
            # unify encodings to cnt_gt-256 (bf16-exact integer in [-256,255])
            rsa = wpool.tile([128, 4], F32, name=f"rsa_{s}", tag="rsa", bufs=4)
            nc.scalar.activation(rsa[:, 0:2], rs[:, 0:2], AF.Copy, bias=-256.0, scale=1.0)
            nc.scalar.activation(rsa[:, 2:4], rs[:, 2:4], AF.Copy, bias=-0.5, scale=0.5)
            expz = wpool.tile([1, 512], F32, name=f"expz_{s}", tag="expz", bufs=2)
            z_sb = wpool.tile([1, 1], F32, name=f"z_{s}", tag="z", bufs=4)
            nc.scalar.activation(expz[:], mv_sb[:], AF.Exp, accum_out=z_sb[:])
            rz = wpool.tile([1, 1], F32, name=f"rz_{s}", tag="rz", bufs=4)
            nc.vector.reciprocal(rz[:], z_sb[:])
            smc = wpool.tile([128, 4], BF16, name=f"smc_{s}", tag="smc", bufs=4)
            nc.scalar.activation(smc[:], mvT_sb[:], AF.Exp)
            code_res[s] = (rsa, rz, smc)

        def emit_wt(s):
            rsa, rz, smc = code_res[s]
            wts = []
            for j in range(4):
                wt = wpool.tile([128, 512], BF16, name=f"wt_{s}_{j}", tag=f"wt{j}", bufs=2)
                nc.vector.tensor_scalar(wt[:], n2bB[:], rsa[:, j:j + 1], None, AL.is_equal)
                wts.append(wt)
            wt_res[s] = wts

        # ---- g row -> broadcast -> gmat tail -> 4-block circulant read ----
        def emit_gchain(s):
            rsa, rz, smc = code_res[s]
            wts = wt_res[s]
            g_ps = psm.tile([1, 512], F32, name=f"gps_{s}", tag="psm", bufs=2)
            for j in range(4):
                nc.tensor.matmul(g_ps[:], smc[:, j:j + 1], wts[j][:], start=(j == 0), stop=(j == 3))
            gn = wpool.tile([1, 512], BF16, name=f"gn_{s}", tag="gn", bufs=2)
            nc.vector.tensor_scalar(gn[:], g_ps[:], rz[:], None, AL.mult)
            gRB = wpool.tile([128, 512], BF16, name=f"gRB_{s}", tag="gRB", bufs=2)
            nc.gpsimd.partition_broadcast(gRB[:], gn[:])
            gmat = dpool.tile([128, 1024], BF16, name=f"gmat_{s}", tag="gmat", bufs=2)
            gd = gmat[:].tensor
            # only cols [384,1024) are read: g at 512:1024, tail g[384:512] at 384:512
            nc.scalar.dma_start(bass.AP(tensor=gd, offset=512, ap=[[1024, 128], [1, 512]]),
                                gRB[:])
            nc.scalar.dma_start(bass.AP(tensor=gd, offset=384, ap=[[1024, 128], [1, 128]]),
                                gRB[:, 384:512])
            cg4 = wpool.tile([128, 512], BF16, name=f"cg4_{s}", tag="cg4", bufs=2)
            nc.scalar.dma_start(cg4[:], bass.AP(tensor=gd, offset=511, ap=[[1023, 128], [1, 512]]))
            chain_res[s] = cg4

        # ---- stage C: block-circulant matmul, 4 distinct stationary blocks ----
        def emit_stagec(s, vsb):
            cg4 = chain_res[s]
            o_sb = wpool.tile([128, 2048], BF16, name=f"osb_{s}", tag="osb", bufs=2)
            for tt in range(4):
                o_ps = pC.tile([128, 512], F32, name=f"ops_{s}_{tt}", tag="ops", bufs=2)
                for ss in range(4):
                    m = (tt - ss) % 4
                    nc.tensor.matmul(o_ps[:], cg4[:, 128 * m:128 * (m + 1)],
                                     vsb[:, 512 * ss:512 * (ss + 1)],
                                     start=(ss == 0), stop=(ss == 3))
                if tt % 2 == 0:
                    nc.scalar.copy(o_sb[:, 512 * tt:512 * (tt + 1)], o_ps[:])
                else:
                    nc.vector.tensor_copy(o_sb[:, 512 * tt:512 * (tt + 1)], o_ps[:])
            nc.sync.dma_start(
                bass.AP(tensor=out_all, offset=s * L * C, ap=[[C, 128], [128 * C, 4], [1, C]]),
                o_sb[:])

        code_res, wt_res, chain_res = {}, {}, {}

        # ================= emission schedule =================
        qk0sb = qpool.tile([128, 4096], BF16, name="qk0sb")
        load_qk_half(qk0_all, 0, qk0sb, 0)
        load_qk_half(qk0_all, 0, qk0sb, 1)

        qksbs = []
        for s in range(NB):
            qksbs.append(qpool.tile([128, 4096], BF16, name=f"qksb_{s}", tag="qkt", bufs=2))
        load_qk_half(qk_all, 0 * 1024 * L, qksbs[0], 0)
        load_qk_half(qk_all, 0 * 1024 * L, qksbs[0], 1)

        M0 = emit_M(qk0sb, "b0", ccs=(0, 1))
        emit_M(qk0sb, "b0", ccs=(2, 3), M_ps=M0)
        load_qk_half(qk_all, 1 * 1024 * L, qksbs[1], 0)
        load_qk_half(qk_all, 1 * 1024 * L, qksbs[1], 1)
        R0 = emit_Pdiag(M0, "b0")

        Ms = [None] * NB
        Ms[0] = emit_M(qksbs[0], "s0")
        load_qk_half(qk_all, 2 * 1024 * L, qksbs[2], 0)
        load_qk_half(qk_all, 2 * 1024 * L, qksbs[2], 1)
        R_s0 = emit_Pdiag(Ms[0], "s0")
        Ms[1] = emit_M(qksbs[1], "s1")
        load_qk_half(qk_all, 3 * 1024 * L, qksbs[3], 0)
        load_qk_half(qk_all, 3 * 1024 * L, qksbs[3], 1)

        # ---- batch-0 rank block ----
        mv0_sb = emit_mv(R0, "b0")
        posmvT = emit_mvT(mv0_sb, "b0")
        mv0r_ps = psm.tile([1, 512], F32, name="mv0r_ps", tag="psm", bufs=2)
        for j in range(4):
            nc.tensor.matmul(mv0r_ps[0:1, 128 * (3 - j):128 * (4 - j)],
                             posmvT[:, j:j + 1], antiI_t, start=True, stop=True)
        mv0r_sb = wpool.tile([1, 512], F32, name="mv0r_sb", bufs=1)
        nc.scalar.copy(mv0r_sb[:], mv0r_ps[:])
        mvB0R = wpool.tile([128, 512], F32, name="mvB0R", bufs=1)
        nc.gpsimd.partition_broadcast(mvB0R[:], mv0r_sb[:])
        r2_ps = psm.tile([1, 512], F32, name="r2_ps", tag="psm", bufs=2)
        for j in range(4):
            c2 = wpool.tile([128, 512], F32R, name=f"c2_{j}", tag="c2", bufs=2)
            nc.gpsimd.tensor_scalar(c2[:], mvB0R[:], posmvT[:, j:j + 1], None, AL.is_lt)
            nc.tensor.matmul(r2_ps[:], two_t, c2[:], start=(j == 0), stop=(j == 3))
        # batch-0 per-position code row: cnt_gt0 - 256 (bf16-exact)
        n2bb_row = wpool.tile([1, 512], BF16, name="n2bb_row", bufs=1)
        nc.scalar.activation(n2bb_row[:], r2_ps[:], AF.Copy, bias=-256.0, scale=0.5)
        n2bB = wpool.tile([128, 512], BF16, name="n2bB", bufs=1)
        nc.gpsimd.partition_broadcast(n2bB[:], n2bb_row[:])

        R_s1 = emit_Pdiag(Ms[1], "s1")
        Ms[2] = emit_M(qksbs[2], "s2")
        vsbs = [load_v(0), load_v(1)]

        mv_s0 = emit_mv(R_s0, "s0")
        mvT_s0 = emit_mvT(mv_s0, "s0")
        finish_code(0, mv_s0, mvT_s0)
        emit_wt(0)

        R_s2 = emit_Pdiag(Ms[2], "s2")
        Ms[3] = emit_M(qksbs[3], "s3")
        vsbs.append(load_v(2))

        mv_s1 = emit_mv(R_s1, "s1")
        mvT_s1 = emit_mvT(mv_s1, "s1")
        finish_code(1, mv_s1, mvT_s1)
        emit_wt(1)
        emit_gchain(0)

        R_s3 = emit_Pdiag(Ms[3], "s3")
        vsbs.append(load_v(3))

        mv_s2 = emit_mv(R_s2, "s2")
        mvT_s2 = emit_mvT(mv_s2, "s2")
        finish_code(2, mv_s2, mvT_s2)
        emit_wt(2)
        emit_gchain(1)

        mv_s3 = emit_mv(R_s3, "s3")
        mvT_s3 = emit_mvT(mv_s3, "s3")
        finish_code(3, mv_s3, mvT_s3)
        emit_wt(3)
        emit_gchain(2)

        emit_stagec(0, vsbs[0])
        emit_gchain(3)
        emit_stagec(1, vsbs[1])
        emit_stagec(2, vsbs[2])
        emit_stagec(3, vsbs[3])

    nc.compile()
    _NC_CACHE = nc
    return nc


def kernel(queries, keys, values):
    q = np.ascontiguousarray(queries, dtype=np.float32).reshape(B, L, C)
    k = np.ascontiguousarray(keys, dtype=np.float32).reshape(B, L, C)
    v = np.ascontiguousarray(values, dtype=np.float32).reshape(B, L, C).astype(BF)
    # [B, C, L] -> [B, cchunk, {q,k}, 128, L]
    qT = np.ascontiguousarray(q.transpose(0, 2, 1)).astype(BF).reshape(B, 4, 128, L)
    kT = np.ascontiguousarray(k.transpose(0, 2, 1)).astype(BF).reshape(B, 4, 128, L)
    qk = np.stack([qT, kT], axis=2)  # [B, 4, 2, 128, L]
    nc = _build()
    in_maps = []
    for c in range(NCORES):
        sl = slice(NB * c, NB * (c + 1))
        in_maps.append({
            "qk_all": qk[sl],
            "qk0_all": qk[0],
            "v_all": v[sl],
        })
    res = run_bass_kernel_spmd(nc, in_maps, core_ids=list(range(NCORES)))
    out = np.concatenate([np.asarray(res.results[c]["out_all"]).astype(np.float32)
                          for c in range(NCORES)], axis=0)
    return out.reshape(B, L, H, E)


if __name__ == "__main__":
    rng = np.random.default_rng(0)
    qq = rng.standard_normal((B, L, H, E)).astype(np.float32)
    kk = rng.standard_normal((B, L, H, E)).astype(np.float32)
    vv = rng.standard_normal((B, L, H, E)).astype(np.float32)
    o = kernel(queries=qq, keys=kk, values=vv)
    print(o.shape, o.dtype, np.abs(o).max())
